# revision 56
# baseline (speedup 1.0000x reference)
"""CapsuleLayer dynamic-routing kernel for 8 Trainium2 NeuronCores.

Problem: x [128, 2048, 8], W [32, 2048, 16, 8] (fp32)
  u_hat[b,j,i,d] = sum_p W[j,i,d,p] * x[b,i,p]
  3 rounds of routing-by-agreement (softmax over j, squash), no
  persistent logits needed: b_k = (sum_{m<k} out_m) . u_hat, so each
  round is a streaming pass over i needing only O_k = sum out_m.

Sharding: i (input capsules) split 8 ways; every core holds the full
batch B=128 on SBUF partitions. Per-round partial sums s[b,(j,d)]
(256KB/core) are reduced on the host between the three launches.
"""

import numpy as np
from contextlib import ExitStack

import concourse.bass as bass
import concourse.mybir as mybir
from concourse import tile
from concourse.bass_utils import run_bass_kernel_spmd

# ---------------------------------------------------------------------------
# Shapes (hardcoded for this problem)
B, I, P = 128, 2048, 8
J, D = 32, 16
JD = J * D               # 512
N_CORES = 8
I_LOC = I // N_CORES     # 256
EPS = 1e-7
GROUP = 4                # i's per routing group (psum tile = GROUP banks)
N_GROUPS = I_LOC // GROUP

_f32 = mybir.dt.float32


# ---------------------------------------------------------------------------
# Walrus compat: this toolchain rejects sync waits on InstDrain and >2 on
# InstEventSemaphore. Emit the waits as standalone nops before the drain.
def _apply_tile_compat():
    from concourse.vector_clock import ScopedClock

    def _strip_waits(inst):
        si = inst.sync_info
        if not si or not si.on_wait:
            return []
        waits = list(si.on_wait)
        si.on_wait = []
        inst.sync_info = si
        return waits

    def _nop_with_wait(eng, w):
        nop = eng.nop(nofuse=True, hint="drain_wait_split")
        nsi = nop.ins.sync_info
        if nsi is None:
            nsi = mybir.SyncInfo(on_wait=[], on_update=[])
        nsi.on_wait = list(nsi.on_wait or []) + [w]
        nop.ins.sync_info = nsi

    def _patched_multi_engine_barrier(self, engines):
        for inst in bass._bass_rust._multi_engine_barrier_insts(
            self, list(engines)
        ):
            eng = self.engines[inst.engine]
            for w in _strip_waits(inst):
                _nop_with_wait(eng, w)
            eng.add_instruction(inst)

    def _patched_drain_and_barrier(self, tick_clock, wait_clock):
        nop_inst = self.nc.sync.nop(nofuse=True, hint="drain_wait_split")
        wait_clock.add_sem_waits(
            nop_inst.ins, ScopedClock({None: tick_clock.global_clock})
        )
        si = nop_inst.ins.sync_info
        if si and si.on_wait and len(si.on_wait) > 1:
            extra = list(si.on_wait[1:])
            si.on_wait = [si.on_wait[0]]
            nop_inst.ins.sync_info = si
            for w in extra:
                _nop_with_wait(self.nc.sync, w)
        self.nc.sync.drain()

        self.nc.all_engine_barrier()
        assert self.sems is not None
        popped = self.nc._tile_sem_poison_stack.pop()
        assert popped is self._sem_poison
        self.nc.clear_and_free_semaphores(list(self.sems.allocated().values()))
        # No trailing all_engine_barrier: every engine is already past the
        # pre-clear barrier (done touching semaphores), nothing reads them
        # afterwards, and NEFF completion only needs each engine to halt.

    # Scheduled body instructions can also end up with >1 wait (e.g. a
    # matmul waiting on two DMAs). Spill extras onto same-engine NoOps
    # inserted immediately before the instruction.
    _WAIT_CAPS = {"InstDrain": 0, "InstEventSemaphore": 2}
    _orig_add_instruction = tile.TileContext._add_instruction

    def _patched_add_instruction(self, inst):
        si = inst.sync_info
        cap = _WAIT_CAPS.get(type(inst).__name__, 1)
        if si and si.on_wait and len(si.on_wait) > cap:
            waits = list(si.on_wait)
            si.on_wait = waits[:cap]
            inst.sync_info = si
            for w in waits[cap:]:
                nop = mybir.InstNoOp(
                    name=f"I-{self.nc.next_id()}-waitspill", ins=[], outs=[]
                )
                nop.engine = inst.engine
                nop.sync_info = mybir.SyncInfo(on_wait=[w], on_update=[])
                _orig_add_instruction(self, nop)
        _orig_add_instruction(self, inst)

    bass.Bass.multi_engine_barrier = _patched_multi_engine_barrier
    tile.TileContext._drain_and_barrier = _patched_drain_and_barrier
    tile.TileContext._add_instruction = _patched_add_instruction


_apply_tile_compat()


# ---------------------------------------------------------------------------
# Launch 1: s0_part[b,(j,d)] = sum_{i local} u_hat[b,j,i,d]
# (iteration 0 has exactly uniform c = 1/32, applied on the host)
def build_l1():
    nc = bass.Bass("TRN2", target_bir_lowering=False, debug=False)
    n_chunks = (I_LOC * P) // 128  # 16
    xw1 = nc.dram_tensor(
        "xw1", [n_chunks, 128, B + JD], _f32, kind="ExternalInput").ap()
    sp = nc.dram_tensor("sp", [B, JD], _f32, kind="ExternalOutput").ap()
    with ExitStack() as ctx:
        tc = ctx.enter_context(tile.TileContext(nc))
        xpool = ctx.enter_context(tc.tile_pool(name="xw1", bufs=4))
        ppool = ctx.enter_context(tc.tile_pool(name="ps", bufs=1, space="PSUM"))
        opool = ctx.enter_context(tc.tile_pool(name="o", bufs=1))
        psum = ppool.tile([B, JD], _f32)
        for q in range(n_chunks):
            t = xpool.tile([128, B + JD], _f32)
            nc.sync.dma_start(t[:], xw1[q])
            nc.tensor.matmul(
                psum[:], lhsT=t[:, :B], rhs=t[:, B:],
                start=(q == 0), stop=(q == n_chunks - 1),
            )
        out = opool.tile([B, JD], _f32)
        nc.scalar.copy(out[:], psum[:])
        nc.sync.dma_start(sp[:], out[:])
    return nc


# ---------------------------------------------------------------------------
# Launches 2 & 3: one routing round.
#   g[b,j,i]  = sum_d O[b,j,d] * u_hat[b,j,i,d]
#   c         = softmax_j(g)
#   s_part    = sum_{i local} c * u_hat
#
# x and W arrive interleaved in 16-i blocks ("xw": per block, the x
# slab [P, 16*B] then the W slab [P, 16*JD], both p-major) so each
# block is one large DMA instead of 16 small ones.
BLK = 16                       # i's per DMA block
N_BLKS = I_LOC // BLK          # 16
XW_X = BLK * B                 # 2048 x columns per block
XW_W = BLK * JD                # 8192 W columns per block
XW_COLS = XW_X + XW_W          # 10240


def build_l2():
    nc = bass.Bass("TRN2", target_bir_lowering=False, debug=False)
    xw = nc.dram_tensor(
        "xw", [N_BLKS, P, XW_COLS], _f32, kind="ExternalInput").ap()
    x2d = nc.dram_tensor("x2", [B, I_LOC * P], _f32, kind="ExternalInput").ap()
    w2d = nc.dram_tensor(
        "w2", [J, D, I_LOC * P], _f32, kind="ExternalInput").ap()
    otd = nc.dram_tensor("ot", [D, J * B], _f32, kind="ExternalInput").ap()
    sp = nc.dram_tensor("sp", [B, JD], _f32, kind="ExternalOutput").ap()

    IP = I_LOC * P  # 2048

    with ExitStack() as ctx:
        tc = ctx.enter_context(tile.TileContext(nc))
        wpool = ctx.enter_context(tc.tile_pool(name="xw", bufs=2))
        tpool = ctx.enter_context(tc.tile_pool(name="tmp", bufs=2))
        gpool = ctx.enter_context(tc.tile_pool(name="g", bufs=2))
        bpool = ctx.enter_context(tc.tile_pool(name="big", bufs=1))
        apool = ctx.enter_context(tc.tile_pool(name="acc", bufs=1))

        # wide accumulators: one GROUP-lane per i-position, reduced once
        # at the end. Two of them so even groups accumulate on DVE and
        # odd groups on GpSimd, halving the DVE add chain.
        s_wide = apool.tile([B, GROUP * JD], _f32)
        nc.gpsimd.memset(s_wide[:], 0.0)
        s_wide2 = apool.tile([B, GROUP * JD], _f32)
        nc.gpsimd.memset(s_wide2[:], 0.0)

        # ---- phase A: g[b,(j,i)] = sum_p x2[b,(i,p)] * (O_j @ W2_j)[b,(i,p)]
        # g_all is reused in place for e = exp(g - m) and then c (softmax
        # numerator / weights): every op is elementwise with identical
        # input/output traversal order.
        g_all = bpool.tile([B, J * I_LOC], _f32)

        with tc.tile_pool(name="vps", bufs=2, space="PSUM") as vppool, \
             tc.tile_pool(name="pa", bufs=1) as papool, \
             tc.tile_pool(name="w2", bufs=2) as w2pool:
            x2 = papool.tile([B, IP], _f32)
            nc.sync.dma_start(x2[:], x2d[:])
            ot = papool.tile([D, J * B], _f32)
            nc.sync.dma_start(ot[:], otd[:])
            for j in range(J):
                w2t = w2pool.tile([D, IP], _f32)
                nc.sync.dma_start(w2t[:], w2d[j])
                vps = vppool.tile([B, IP], _f32)
                for q in range(IP // 512):
                    nc.tensor.matmul(
                        vps[:, q * 512:(q + 1) * 512],
                        lhsT=ot[:, j * B:(j + 1) * B],
                        rhs=w2t[:, q * 512:(q + 1) * 512],
                        start=True, stop=True,
                    )
                xv = tpool.tile([B, IP], _f32)
                nc.vector.tensor_tensor(
                    xv[:], x2[:], vps[:], op=mybir.AluOpType.mult,
                )
                nc.vector.reduce_sum(
                    g_all[:, j * I_LOC:(j + 1) * I_LOC],
                    xv[:].rearrange("b (i p) -> b i p", i=I_LOC, p=P),
                    axis=mybir.AxisListType.X,
                )

        # ---- softmax over j (free-dim strided, one shot for all i).
        # No max-subtraction: g = O.u_hat with squashed O (|O_j| < 1) is
        # bounded well inside exp's fp32 range, and softmax is shift-
        # invariant, so exp(g)/sum exp(g) matches the reference exactly.
        gjv = g_all[:].rearrange("b (j i) -> b j i", j=J, i=I_LOC)
        giv = g_all[:].rearrange("b (j i) -> b i j", j=J, i=I_LOC)
        nc.scalar.activation(
            g_all[:], g_all[:], mybir.ActivationFunctionType.Exp
        )
        Z = bpool.tile([B, I_LOC], _f32)
        nc.vector.reduce_sum(Z[:], giv, axis=mybir.AxisListType.X)
        Zr = bpool.tile([B, I_LOC], _f32)
        nc.vector.reciprocal(Zr[:], Z[:])
        nc.vector.tensor_tensor(
            gjv, gjv, Zr[:].unsqueeze(1).broadcast_to([B, J, I_LOC]),
            op=mybir.AluOpType.mult,
        )
        c_v = giv

        # ---- phase B: s += sum_i c * u_hat, u_hat recomputed per group.
        # The weighted tiles w are accumulated on the PE into a resident
        # PSUM region via identity matmuls (start=False), so the DVE only
        # does the c-multiply.
        ppool = ctx.enter_context(tc.tile_pool(name="ps", bufs=2, space="PSUM"))
        xw_tiles = {}
        for gi in range(N_GROUPS):
            blk, sub = divmod(gi * GROUP, BLK)
            if sub == 0:
                xwt = wpool.tile([P, XW_COLS], _f32)
                nc.sync.dma_start(xwt[:], xw[blk])
                xw_tiles[blk] = xwt
            xwt = xw_tiles[blk]
            psum = ppool.tile([B, GROUP * JD], _f32)
            for t in range(GROUP):
                ib = sub + t           # i index within the block
                nc.tensor.matmul(
                    psum[:, t * JD:(t + 1) * JD],
                    lhsT=xwt[:, ib * B:(ib + 1) * B],
                    rhs=xwt[:, XW_X + ib * JD:XW_X + (ib + 1) * JD],
                    start=True, stop=True,
                )
            pv = psum[:].rearrange("b (i j d) -> b i j d", i=GROUP, j=J, d=D)
            cslice = c_v[:, gi * GROUP:(gi + 1) * GROUP, :]
            w = tpool.tile([B, GROUP * JD], _f32)
            wv = w[:].rearrange("b (i j d) -> b i j d", i=GROUP, j=J, d=D)
            nc.vector.tensor_tensor(
                wv, pv, cslice.unsqueeze(3).broadcast_to([B, GROUP, J, D]),
                op=mybir.AluOpType.mult,
            )
            if gi % 2 == 0:
                nc.vector.tensor_add(s_wide[:], s_wide[:], w[:])
            else:
                nc.gpsimd.tensor_add(s_wide2[:], s_wide2[:], w[:])

        nc.vector.tensor_add(s_wide[:], s_wide[:], s_wide2[:])
        s_acc = gpool.tile([B, JD], _f32)
        nc.vector.reduce_sum(
            s_acc[:],
            s_wide[:].rearrange("b (i jd) -> b jd i", i=GROUP, jd=JD),
            axis=mybir.AxisListType.X,
        )
        nc.sync.dma_start(sp[:], s_acc[:])
    return nc


# ---------------------------------------------------------------------------
# Host glue
def _squash(s):
    v = s.reshape(B, J, D)
    s2 = np.sum(np.square(v), axis=-1, keepdims=True)
    scale = s2 / (1.0 + s2) / np.sqrt(s2 + EPS)
    return (scale * v).astype(np.float32)


_cache = {}


def _get_nc(name):
    if name not in _cache:
        _cache[name] = build_l1() if name == "l1" else build_l2()
    return _cache[name]


def _prep_inputs(x, W):
    """Per-core host-side re-layouts (all fp32, cheap transposes)."""
    per_core = []
    for c in range(N_CORES):
        sl = slice(c * I_LOC, (c + 1) * I_LOC)
        xc = x[:, sl, :]                                   # [B, I_LOC, P]
        wc = W[:, sl, :, :]                                # [J, I_LOC, D, P]
        xp = np.ascontiguousarray(
            xc.transpose(1, 2, 0).reshape(I_LOC * P, B))   # (i,p),b
        wt = np.ascontiguousarray(
            wc.transpose(1, 3, 0, 2).reshape(I_LOC * P, JD))  # (i,p),(j,d)
        # L1: interleave x/W per 128-row chunk so each chunk is one DMA
        n_chunks = (I_LOC * P) // 128
        xw1 = np.empty((n_chunks, 128, B + JD), np.float32)
        xw1[:, :, :B] = xp.reshape(n_chunks, 128, B)
        xw1[:, :, B:] = wt.reshape(n_chunks, 128, JD)
        # interleaved blocks for L2/L3: per 16-i block, [P, 16*B | 16*JD]
        xb = xc.transpose(2, 1, 0).reshape(P, N_BLKS, BLK * B)  # p,(blk,i*b)
        wb = wc.transpose(3, 1, 0, 2).reshape(P, N_BLKS, BLK, JD)
        xw = np.empty((N_BLKS, P, XW_COLS), np.float32)
        xw[:, :, :XW_X] = xb.transpose(1, 0, 2)
        xw[:, :, XW_X:] = wb.transpose(1, 0, 2, 3).reshape(N_BLKS, P, XW_W)
        # V-trick layouts
        x2 = np.ascontiguousarray(xc.reshape(B, I_LOC * P))      # b,(i,p)
        w2 = np.ascontiguousarray(
            wc.transpose(0, 2, 1, 3).reshape(J, D, I_LOC * P))   # j,d,(i,p)
        per_core.append({"xw1": xw1, "xw": xw, "x2": x2, "w2": w2})
    return per_core


def _ot_layout(O):
    """O [B, JD] -> lhsT layout [D, J*B] for the V matmuls."""
    return np.ascontiguousarray(
        O.reshape(B, J, D).transpose(2, 1, 0).reshape(D, J * B))


def _run(nc, in_maps, **kw):
    res = run_bass_kernel_spmd(nc, in_maps, list(range(N_CORES)), **kw)
    return res


def kernel(x, W, _collect_times=None):
    x = np.asarray(x, dtype=np.float32)
    W = np.asarray(W, dtype=np.float32)
    pc = _prep_inputs(x, W)

    nc1 = _get_nc("l1")
    nc2 = _get_nc("l2")

    r1 = _run(nc1, [{"xw1": p["xw1"]} for p in pc])
    s0 = np.sum([r1.results[c]["sp"] for c in range(N_CORES)], axis=0)
    s0 *= (1.0 / J)
    out0 = _squash(s0)
    O1 = out0.reshape(B, JD)

    ot1 = _ot_layout(O1)
    r2 = _run(nc2, [
        {"xw": p["xw"], "x2": p["x2"], "w2": p["w2"], "ot": ot1}
        for p in pc
    ])
    s1 = np.sum([r2.results[c]["sp"] for c in range(N_CORES)], axis=0)
    out1 = _squash(s1)
    O2 = (out0 + out1).reshape(B, JD)

    ot2 = _ot_layout(O2)
    r3 = _run(nc2, [
        {"xw": p["xw"], "x2": p["x2"], "w2": p["w2"], "ot": ot2}
        for p in pc
    ])
    s2 = np.sum([r3.results[c]["sp"] for c in range(N_CORES)], axis=0)
    out2 = _squash(s2)

    if _collect_times is not None:
        for r in (r1, r2, r3):
            _collect_times.append(r.exec_time_ns)
    return out2
